# revision 2
# baseline (speedup 1.0000x reference)
"""Causal single-head attention on 8 Trainium2 NeuronCores — v5.

Problem: B=4, S=2048, D_IN=1024, D_OUT=64 (fp32).
  Q = Xq @ Wq; K = Xk @ Wk; V = Xv @ Wv
  out = softmax(mask(Q K^T / 8)) @ V

Sharding: 8 cores = 4 batches x 2 interleaved query-block sets (core
c: batch c//2, q blocks g = 2t + (c%2)); K/V rows split contiguously,
projected locally, exchanged via one 2-core AllGather.

Cost model measured on this runtime: every PE matmult ~45us and every
InstLdweights ~34us FLAT regardless of shape/dtype/weight reuse; any
other engine instruction ~14us + size term; DMA ~0.6us/descriptor;
sem updates free; carried waits ~15us. v5 therefore minimizes
instruction count:
  - scores computed only over each k-tile's causal q-range using ONE
    parity-independent table (the wider h=1 ranges): 24 matmuls vs 52.
    No bias matmuls: boundary-tile masking is data-driven — the first
    128-col block of every k-tile range is multiplied by a per-core
    mask (tril / zeros / ones) in just TWO strided-AP DVE multiplies
    (even-kti blocks at elem offset 2176*j, odd at 1024+2176*j — both
    regular APs over the padded expt buffer),
  - redundant InstLdweights dropped post-legalization when consecutive
    matmults share the identical stationary operand (48 removed),
  - K and V projections share one [Wk|Wv] stationary tile (8 loads);
    Q rides in the K unit's partitions 64:128 as a sequential group,
  - AV accumulates into one [65,1024] PSUM tile with nested col
    ranges, kti ascending, start only on the first; score units + exps
    run in REVERSE order so AV's first matmul carries the only Act
    wait (vector clocks drop the rest),
  - output is the raw [65, SC] numerator/denominator block; divide +
    transpose on host,
  - X shipped bf16, pre-transposed AND pre-partition-packed host-side
    so the single input DMA is 128 contiguous descriptors.
"""

import os
import numpy as np
import ml_dtypes

import concourse.bass as bass
import concourse.mybir as mybir
import concourse.tile as tile
from concourse.bass_utils import run_bass_kernel_spmd
from concourse.vector_clock import ScopedClock

# ---------------------------------------------------------------------------
# Workaround: the walrus in this container rejects Tile's end-of-kernel drain
# when it carries >1 sem wait ("Too many sync wait commands").
# ---------------------------------------------------------------------------


def _patched_drain_and_barrier(self, tick_clock, wait_clock):
    nc = self.nc
    collector = nc.sync.nop(nofuse=True)
    wait_clock.add_sem_waits(
        collector.ins, ScopedClock({None: tick_clock.global_clock})
    )
    si = collector.ins.sync_info
    waits = list(si.on_wait or []) if si is not None else []
    if si is not None:
        si.on_wait = waits[:1]
    for w in waits[1:]:
        n = nc.sync.nop(nofuse=True)
        nsi = n.ins.sync_info
        if nsi is None:
            n.ins.sync_info = mybir.SyncInfo(on_wait=[w], on_update=[])
        else:
            nsi.on_wait = [w]
    nc.sync.drain()
    nc.all_engine_barrier()
    assert self.sems is not None
    popped = nc._tile_sem_poison_stack.pop()
    assert popped is self._sem_poison
    nc.clear_and_free_semaphores(list(self.sems.allocated().values()))
    nc.all_engine_barrier()


tile.TileContext._drain_and_barrier = _patched_drain_and_barrier


def _split_sync_waits(nc, limit=1):
    """The nix walrus allows only `limit` sem waits per instruction; hoist
    extras onto same-engine NOPs placed immediately before the instruction."""
    ctr = [0]
    for fn in nc.m.functions:
        for bb in fn.blocks:
            out_list = []
            changed = False
            for inst in bb.instructions:
                si = inst.sync_info
                waits = list(si.on_wait) if si is not None and si.on_wait else []
                if len(waits) > limit:
                    keep = waits[-limit:]
                    for w in waits[:-limit]:
                        ctr[0] += 1
                        nop = mybir.InstNoOp(
                            name=f"waitsplit-{ctr[0]}",
                            engine=inst.engine,
                            ins=[],
                            outs=[],
                            sync_info=mybir.SyncInfo(on_wait=[w], on_update=[]),
                        )
                        out_list.append(nop)
                    si.on_wait = keep
                    changed = True
                out_list.append(inst)
            if changed:
                bb.instructions = out_list


def _ap_sig(pap):
    return (
        pap.memref,
        pap.offset,
        tuple(tuple(x) for x in pap.ap),
        str(pap.dtype),
    )


def _dedup_ldweights(nc):
    """Drop InstLdweights that reload the identical stationary operand with
    no intervening PE-array clobber. Waits move to the next kept PE
    instruction (ldweights carry no sem updates, so counts are safe)."""
    removed = 0
    for fn in nc.m.functions:
        for bb in fn.blocks:
            out = []
            last_sig = None
            pending = []
            for inst in bb.instructions:
                if not str(inst.engine).endswith("PE"):
                    out.append(inst)
                    continue
                tname = type(inst).__name__
                if tname == "InstLdweights":
                    sig = (
                        _ap_sig(inst.ins[0]),
                        str(getattr(inst, "perf_mode", None)),
                        str(getattr(inst, "is_transpose", None)),
                        str(getattr(inst, "tile_position", None)),
                        str(getattr(inst, "tile_size", None)),
                    )
                    si = inst.sync_info
                    if sig == last_sig and not (si is not None and si.on_update):
                        if si is not None and si.on_wait:
                            pending.extend(si.on_wait)
                        removed += 1
                        continue
                    last_sig = sig
                elif tname == "InstMatmult":
                    if getattr(inst, "is_transpose", None):
                        last_sig = None
                else:
                    last_sig = None
                if pending:
                    si = inst.sync_info
                    if si is None:
                        inst.sync_info = mybir.SyncInfo(
                            on_wait=list(pending), on_update=[]
                        )
                    else:
                        si.on_wait = list(pending) + list(si.on_wait or [])
                    pending = []
                out.append(inst)
            assert not pending
            bb.instructions = out
    return removed


def _prune_waits(nc):
    """Remove waits that are (a) transitively guaranteed by an earlier wait on
    the same engine, or (b) on a semaphore whose required count is already met
    by the engine's OWN prior updates (in-order engines satisfy these by
    program order). Barrier (eq-imm) waits are never touched."""
    removed = 0
    own = {}         # engine -> {sem_id: count of own updates emitted so far}
    guaranteed = {}  # engine -> {sem_id: max value already waited-for}
    for fn in nc.m.functions:
        for bb in fn.blocks:
            for inst in bb.instructions:
                eng = str(inst.engine)
                g = guaranteed.setdefault(eng, {})
                o = own.setdefault(eng, {})
                si = inst.sync_info
                if si is not None and si.on_wait:
                    keep = []
                    for w in si.on_wait:
                        if w.wait_mode != "sem-ge-imm" or "barrier" in (
                            w.ant_name or ""
                        ):
                            keep.append(w)
                            # eq-imm barrier resets guarantees for this sem
                            g.pop(w.id, None)
                            continue
                        if g.get(w.id, -1) >= w.wait_value:
                            removed += 1
                            continue
                        if o.get(w.id, 0) >= w.wait_value:
                            removed += 1
                            g[w.id] = max(g.get(w.id, -1), w.wait_value)
                            continue
                        g[w.id] = w.wait_value
                        keep.append(w)
                    si.on_wait = keep
                if si is not None and si.on_update:
                    eshort = eng.split(".")[-1] + "_"
                    for u in si.on_update:
                        # only count updates on the engine's OWN queue sem:
                        # DMA/collective completions are async and must not
                        # be treated as satisfied-by-program-order
                        if u.update_mode == "sem-inc" and (
                            u.ant_name or ""
                        ).startswith(eshort):
                            o[u.id] = o.get(u.id, 0) + (u.update_value or 1)
    return removed


# ---------------------------------------------------------------------------

B, S, D, E = 4, 2048, 1024, 64
SC = S // 2          # query/kv rows per core
NT = SC // 128       # 8 local query blocks
NKT = S // 16 // 8   # 16 k-tiles
NKT = S // 128
ND = D // 128        # 8 d-tiles
E1 = E + 1           # V columns + ones column
PITCH = 2 * SC + 128  # 2176: elem pitch of same-parity first-blocks in expt
EPLEN = SC + 8 * PITCH  # padded expt length so both strided views are in-AP

F32 = mybir.dt.float32
BF16 = mybir.dt.bfloat16
EXP = mybir.ActivationFunctionType.Exp

KERNEL_UID = 7000

# parity-independent causal table: t0[kti] = ceil((kti-1)/2) (h=1 ranges,
# a superset of h=0's; surplus is zeroed by the data-driven masks)
T0 = {kti: max(0, -(-(kti - 1) // 2)) for kti in range(NKT)}
T0 = {k: t for k, t in T0.items() if t * 128 < SC}


def _units():
    """Score units: one kti if W>512 else a pair of equal-width ktis."""
    units = []
    ks = sorted(T0)
    i = 0
    while i < len(ks):
        k = ks[i]
        W = SC - T0[k] * 128
        if W > 512 or i + 1 >= len(ks):
            units.append(((k,), T0[k], W))
            i += 1
        else:
            k2 = ks[i + 1]
            assert SC - T0[k2] * 128 == W, (k, k2)
            units.append(((k, k2), T0[k], W))
            i += 2
    return units


UNITS = _units()
NU = len(UNITS)


def _build_nc(loop_reps=None, split=True, timing_mode=False, uid=KERNEL_UID):
    nc = bass.Bass()

    if timing_mode:
        xall = nc.dram_tensor("xall", (128, ND * 3 * SC), BF16)
    else:
        xall = nc.dram_tensor("xall", (128, ND * 3 * SC), BF16,
                              kind="ExternalInput")
    # wall: [128, ND*192]: per (p, a): cols 0:128 = [Wk|Wv][a*128+p, :],
    # 128:192 = Wq[a*128+p, :]
    wall = nc.dram_tensor("wall", (128, ND * 192), BF16, kind="ExternalInput")
    # per-core masks for the first 128-col block of each k-tile range:
    # maskin[0] applies to even ktis, maskin[1] to odd ktis
    maskin = nc.dram_tensor("maskin", (2, 128, 128), BF16, kind="ExternalInput")
    nc.dram_tensor("vtag", (1, uid), F32, kind="ExternalInput")
    out = nc.dram_tensor("out", (E1, SC), F32, kind="ExternalOutput")

    with tile.TileContext(nc) as tc:
        with (
            tc.tile_pool(name="const", bufs=1) as cpool,
            tc.tile_pool(name="ps_u", bufs=3, space="PSUM") as ps_u,
            tc.tile_pool(name="ps_av", bufs=1, space="PSUM") as ps_av,
            tc.tile_pool(name="dram", bufs=1, space="DRAM") as dpool,
        ):
            # ---- one-time constants ----
            w_all = cpool.tile([128, ND, 192], BF16, tag="w_all")
            nc.sync.dma_start(
                out=w_all[:],
                in_=wall[:, :].rearrange("p (a e) -> p a e", e=192),
            )
            w_kv = w_all[:, :, 0:128]
            w_q = w_all[:, :, 128:192]
            masks_sb = cpool.tile([128, 2, 128], BF16, tag="mask")
            nc.sync.dma_start(
                out=masks_sb[:], in_=maskin[:, :, :].rearrange("m p q -> p m q")
            )

            if timing_mode:
                zt = cpool.tile([128, ND * 3 * SC], BF16, tag="zt")
                nc.vector.memset(zt[:], 0.0)
                nc.sync.dma_start(out=xall[:, :], in_=zt[:])

            def init_body(par):
                v1 = cpool.tile([128, NKT, E1], BF16, tag=f"v1{par}")
                ep = cpool.tile([128, EPLEN], BF16, tag=f"ep{par}")
                nc.gpsimd.memset(v1[:, :, E : E + 1], 1.0)
                nc.gpsimd.memset(ep[:], 0.0)

            init_body(0)
            init_body(1)

            def emit_body(par):
                xT = cpool.tile([128, ND, 3, SC], BF16, tag="xT")
                kt = cpool.tile([E, S], BF16, tag=f"kt{par}")
                qt = cpool.tile([E, SC], BF16, tag=f"qt{par}")
                kvt = cpool.tile([128, SC], BF16, tag=f"kvt{par}")
                v1 = cpool.tile([128, NKT, E1], BF16, tag=f"v1{par}")
                ep = cpool.tile([128, EPLEN], BF16, tag=f"ep{par}")
                expt = ep[:, 0 : NKT * SC].rearrange("p (k q) -> p k q", q=SC)
                avsb = cpool.tile([E1, SC], F32, tag=f"avsb{par}")

                # ---- X^T load (host pre-packed; contiguous per partition)
                nc.sync.dma_start(
                    out=xT[:],
                    in_=xall[:, :].rearrange("p (a j s) -> p a j s", a=ND, j=3),
                )

                # ---- projections ----
                # pps_k: parts 0:64 = K^T (64:128 garbage, then overwritten
                # by the Q group); pps_v: parts 64:128 = V^T (0:64 garbage).
                pps_k = ps_u.tile([128, 1024], F32, tag="u")
                pps_v = ps_u.tile([128, 1024], F32, tag="u")
                for dt in range(ND):
                    for c in range(2):
                        nc.tensor.matmul(
                            pps_k[:, c * 512 : (c + 1) * 512],
                            w_kv[:, dt, :],
                            xT[:, dt, 0, c * 512 : (c + 1) * 512],
                            start=(dt == 0), stop=(dt == ND - 1),
                            skip_group_check=True,
                        )
                    for c in range(2):
                        nc.tensor.matmul(
                            pps_v[:, c * 512 : (c + 1) * 512],
                            w_kv[:, dt, :],
                            xT[:, dt, 2, c * 512 : (c + 1) * 512],
                            start=(dt == 0), stop=(dt == ND - 1),
                            skip_group_check=True,
                        )
                nc.scalar.copy(out=kvt[E : 2 * E, :], in_=pps_v[E : 2 * E, :])
                # Q as a sequential group into pps_k parts 64:128 (after all
                # K matmuls; start=True re-zeroes the garbage there)
                for dt in range(ND):
                    for c in range(2):
                        nc.tensor.matmul(
                            pps_k[E : 2 * E, c * 512 : (c + 1) * 512],
                            w_q[:, dt, :],
                            xT[:, dt, 1, c * 512 : (c + 1) * 512],
                            start=(dt == 0), stop=(dt == ND - 1),
                            skip_group_check=True,
                        )
                nc.scalar.copy(out=kvt[0:E, :], in_=pps_k[0:E, :])
                nc.scalar.copy(out=qt[:], in_=pps_k[E : 2 * E, :])

                # ---- K/V exchange: one pairwise AllGather ----
                src_kv = dpool.tile([128, SC], BF16, tag=f"cc_src{par}")
                dst_kv = dpool.tile([2, 128, SC], BF16, tag=f"cc_dst{par}")
                nc.scalar.dma_start(out=src_kv[:, :], in_=kvt[:])
                nc.gpsimd.collective_compute(
                    "AllGather",
                    mybir.AluOpType.bypass,
                    replica_groups=[[0, 1], [2, 3], [4, 5], [6, 7]],
                    ins=[src_kv[:]],
                    outs=[dst_kv[:]],
                )
                nc.gpsimd.dma_start(
                    out=kt[:].rearrange("e (r s) -> e r s", r=2),
                    in_=dst_kv[:, 0:E, :].rearrange("r e s -> e r s"),
                )
                # V natural via 2 XBAR transposes + copies into v1
                vscr = cpool.tile([128, 2, 8, E], BF16, tag=f"vscr{par}")
                for r in range(2):
                    nc.sync.dma_start_transpose(
                        out=vscr[:, r, :, :],
                        in_=dst_kv[r, E : 2 * E, :],
                    )
                for r in (1, 0):
                    nc.gpsimd.tensor_copy(
                        out=v1[:, r * 8 : (r + 1) * 8, 0:E], in_=vscr[:, r]
                    )

                # ---- scores + exp, REVERSE unit order ----
                def unit_scores(u):
                    ktis, t0, W = UNITS[u]
                    sps = ps_u.tile([128, 1024], F32, tag="u")
                    base = 0 if len(ktis) == 1 else 512 - W
                    for j, kti in enumerate(ktis):
                        lo = base + j * W
                        off = 0
                        while off < W:
                            w_ = min(512, W - off)
                            nc.tensor.matmul(
                                sps[:, lo + off : lo + off + w_],
                                kt[:, kti * 128 : (kti + 1) * 128],
                                qt[:, t0 * 128 + off : t0 * 128 + off + w_],
                                start=True, stop=True,
                                skip_group_check=True,
                            )
                            off += w_
                    return sps

                def unit_exp(u, sps):
                    ktis, t0, W = UNITS[u]
                    k0 = ktis[0]
                    nk = len(ktis)
                    base = 0 if nk == 1 else 512 - W
                    nc.scalar.activation(
                        expt[:, k0 : k0 + nk, t0 * 128 : t0 * 128 + W],
                        sps[:, base : base + nk * W].rearrange(
                            "p (j q) -> p j q", j=nk
                        ),
                        EXP,
                    )

                batches = []
                us = list(range(NU - 1, -1, -1))
                for i in range(0, NU, 3):
                    batches.append(us[i : i + 3])
                for batch in batches:
                    tiles = [(u, unit_scores(u)) for u in batch]
                    for u, sps in reversed(tiles):
                        unit_exp(u, sps)

                # ---- data-driven boundary masks: 2 strided DVE multiplies
                # even ktis 2j: first block at elem offset j*PITCH
                # odd ktis 2j+1: first block at elem offset SC + j*PITCH
                for m, base in ((1, SC), (0, 0)):  # odd first, even LAST
                    blk = (
                        ep[:, base : base + 8 * PITCH]
                        .rearrange("p (t r) -> p t r", r=PITCH)[:, :, 0:128]
                    )
                    nc.vector.tensor_mul(
                        blk,
                        blk,
                        masks_sb[:, m : m + 1, :].broadcast_to([128, 8, 128]),
                    )

                # ---- AV: one [65,1024] psum, kti ASCENDING, nested ranges
                av = ps_av.tile([E1, SC], F32, tag="av")
                ks = sorted(T0)
                for i, kti in enumerate(ks):
                    lo = T0[kti] * 128
                    first, last = i == 0, i == len(ks) - 1
                    if lo < 512:
                        nc.tensor.matmul(
                            av[:, lo:512],
                            v1[:, kti, :],
                            expt[:, kti, lo:512],
                            start=first, stop=last,
                            skip_group_check=True,
                        )
                    nc.tensor.matmul(
                        av[:, 512:SC],
                        v1[:, kti, :],
                        expt[:, kti, 512:SC],
                        start=first, stop=last,
                        skip_group_check=True,
                    )

                # ---- store numerator+denominator; host divides ----
                nc.vector.tensor_copy(out=avsb[:], in_=av[:])
                nc.sync.dma_start(out=out[:, :], in_=avsb[:])

            for _rep in range(1 if loop_reps is None else loop_reps):
                emit_body(_rep % 2)

    _dedup_ldweights(nc)
    _prune_waits(nc)
    if split:
        _split_sync_waits(nc)
    return nc


_CACHE = {}


def _get_nc():
    if "nc" not in _CACHE:
        _CACHE["nc"] = _build_nc()
    return _CACHE["nc"]


def _host_masks(h):
    """maskin[0] -> even ktis, maskin[1] -> odd ktis. Same-parity ktis get
    the tril boundary mask; other-parity ktis get all-ones for h=1 (their
    first block is fully causal there) or all-zeros for h=0 (fully above
    the diagonal)."""
    ki = np.arange(128)[:, None]
    qi = np.arange(128)[None, :]
    tril = (ki <= qi).astype(np.float32)
    other = np.full((128, 128), 1.0 if h == 1 else 0.0, np.float32)
    m = np.empty((2, 128, 128), np.float32)
    m[h] = tril
    m[1 - h] = other
    return m.astype(ml_dtypes.bfloat16)


def kernel(**inputs):
    xq_full = np.asarray(inputs["inputs_for_queries"], dtype=np.float32).astype(
        ml_dtypes.bfloat16
    )
    xk_full = np.asarray(inputs["inputs_for_keys"], dtype=np.float32).astype(
        ml_dtypes.bfloat16
    )
    xv_full = np.asarray(inputs["inputs_for_values"], dtype=np.float32).astype(
        ml_dtypes.bfloat16
    )
    wq = (np.asarray(inputs["Weight_Q"], dtype=np.float32) * 0.125).astype(
        ml_dtypes.bfloat16
    )
    wk = np.asarray(inputs["Weight_K"], dtype=np.float32).astype(ml_dtypes.bfloat16)
    wv = np.asarray(inputs["Weight_V"], dtype=np.float32).astype(ml_dtypes.bfloat16)
    wkv = np.concatenate([wk, wv], axis=1)  # [D, 128]
    wall = np.concatenate(
        [wkv.reshape(ND, 128, 128), wq.reshape(ND, 128, 64)], axis=2
    )
    wall = np.ascontiguousarray(wall.transpose(1, 0, 2).reshape(128, ND * 192))

    masks_h = [_host_masks(h) for h in (0, 1)]

    nc = _get_nc()

    in_maps = []
    for c in range(8):
        b, h = c // 2, c % 2
        rows = np.concatenate(
            [np.arange((2 * t + h) * 128, (2 * t + h + 1) * 128) for t in range(NT)]
        )
        xk_h = xk_full[b][h * SC : (h + 1) * SC]  # [SC, D]
        xq_h = xq_full[b][rows]
        xv_h = xv_full[b][h * SC : (h + 1) * SC]
        xs = np.stack(
            [x.T.reshape(ND, 128, SC) for x in (xk_h, xq_h, xv_h)], axis=2
        )  # [ND, 128, 3, SC]
        xall = np.ascontiguousarray(
            xs.transpose(1, 0, 2, 3).reshape(128, ND * 3 * SC)
        )
        in_maps.append(
            {
                "xall": xall,
                "wall": wall,
                "maskin": masks_h[h],
                "vtag": np.zeros((1, KERNEL_UID), np.float32),
            }
        )

    trace = bool(int(os.environ.get("KERNEL_TRACE", "0")))
    res = run_bass_kernel_spmd(nc, in_maps, core_ids=list(range(8)), trace=trace)
    if trace:
        _CACHE["last_results"] = res

    out_full = np.empty((B, S, E), dtype=np.float32)
    for c in range(8):
        b, h = c // 2, c % 2
        oc = res.results[c]["out"]  # [65, SC]
        numer = oc[0:E, :]
        den = oc[E, :]
        for t in range(NT):
            g = 2 * t + h
            sl = slice(t * 128, (t + 1) * 128)
            out_full[b, g * 128 : (g + 1) * 128] = (numer[:, sl] / den[sl]).T
    return out_full


# revision 3
# speedup vs baseline: 1.0750x; 1.0750x over previous
"""Causal single-head attention on 8 Trainium2 NeuronCores — v5.

Problem: B=4, S=2048, D_IN=1024, D_OUT=64 (fp32).
  Q = Xq @ Wq; K = Xk @ Wk; V = Xv @ Wv
  out = softmax(mask(Q K^T / 8)) @ V

Sharding: 8 cores = 4 batches x 2 interleaved query-block sets (core
c: batch c//2, q blocks g = 2t + (c%2)); K/V rows split contiguously,
projected locally, exchanged via one 2-core AllGather.

Cost model measured on this runtime: every PE matmult ~45us and every
InstLdweights ~34us FLAT regardless of shape/dtype/weight reuse; any
other engine instruction ~14us + size term; DMA ~0.6us/descriptor;
sem updates free; carried waits ~15us. v5 therefore minimizes
instruction count:
  - scores computed only over each k-tile's causal q-range using ONE
    parity-independent table (the wider h=1 ranges): 24 matmuls vs 52.
    No bias matmuls: boundary-tile masking is data-driven — the first
    128-col block of every k-tile range is multiplied by a per-core
    mask (tril / zeros / ones) in just TWO strided-AP DVE multiplies
    (even-kti blocks at elem offset 2176*j, odd at 1024+2176*j — both
    regular APs over the padded expt buffer),
  - redundant InstLdweights dropped post-legalization when consecutive
    matmults share the identical stationary operand (48 removed),
  - K and V projections share one [Wk|Wv] stationary tile (8 loads);
    Q rides in the K unit's partitions 64:128 as a sequential group,
  - AV accumulates into one [65,1024] PSUM tile with nested col
    ranges, kti ascending, start only on the first; score units + exps
    run in REVERSE order so AV's first matmul carries the only Act
    wait (vector clocks drop the rest),
  - output is the raw [65, SC] numerator/denominator block; divide +
    transpose on host,
  - X shipped bf16, pre-transposed AND pre-partition-packed host-side
    so the single input DMA is 128 contiguous descriptors.
"""

import os
import numpy as np
import ml_dtypes

import concourse.bass as bass
import concourse.mybir as mybir
import concourse.tile as tile
from concourse.bass_utils import run_bass_kernel_spmd
from concourse.vector_clock import ScopedClock

# ---------------------------------------------------------------------------
# Workaround: the walrus in this container rejects Tile's end-of-kernel drain
# when it carries >1 sem wait ("Too many sync wait commands").
# ---------------------------------------------------------------------------


def _patched_drain_and_barrier(self, tick_clock, wait_clock):
    nc = self.nc
    collector = nc.sync.nop(nofuse=True)
    wait_clock.add_sem_waits(
        collector.ins, ScopedClock({None: tick_clock.global_clock})
    )
    si = collector.ins.sync_info
    waits = list(si.on_wait or []) if si is not None else []
    if si is not None:
        si.on_wait = waits[:1]
    for w in waits[1:]:
        n = nc.sync.nop(nofuse=True)
        nsi = n.ins.sync_info
        if nsi is None:
            n.ins.sync_info = mybir.SyncInfo(on_wait=[w], on_update=[])
        else:
            nsi.on_wait = [w]
    nc.sync.drain()
    nc.all_engine_barrier()
    assert self.sems is not None
    popped = nc._tile_sem_poison_stack.pop()
    assert popped is self._sem_poison
    nc.clear_and_free_semaphores(list(self.sems.allocated().values()))
    nc.all_engine_barrier()


tile.TileContext._drain_and_barrier = _patched_drain_and_barrier


def _split_sync_waits(nc, limit=1):
    """The nix walrus allows only `limit` sem waits per instruction; hoist
    extras onto same-engine NOPs placed immediately before the instruction."""
    ctr = [0]
    for fn in nc.m.functions:
        for bb in fn.blocks:
            out_list = []
            changed = False
            for inst in bb.instructions:
                si = inst.sync_info
                waits = list(si.on_wait) if si is not None and si.on_wait else []
                if len(waits) > limit:
                    keep = waits[-limit:]
                    for w in waits[:-limit]:
                        ctr[0] += 1
                        nop = mybir.InstNoOp(
                            name=f"waitsplit-{ctr[0]}",
                            engine=inst.engine,
                            ins=[],
                            outs=[],
                            sync_info=mybir.SyncInfo(on_wait=[w], on_update=[]),
                        )
                        out_list.append(nop)
                    si.on_wait = keep
                    changed = True
                out_list.append(inst)
            if changed:
                bb.instructions = out_list


def _ap_sig(pap):
    return (
        pap.memref,
        pap.offset,
        tuple(tuple(x) for x in pap.ap),
        str(pap.dtype),
    )


def _dedup_ldweights(nc):
    """Drop InstLdweights that reload the identical stationary operand with
    no intervening PE-array clobber. Waits move to the next kept PE
    instruction (ldweights carry no sem updates, so counts are safe)."""
    removed = 0
    for fn in nc.m.functions:
        for bb in fn.blocks:
            out = []
            last_sig = None
            pending = []
            for inst in bb.instructions:
                if not str(inst.engine).endswith("PE"):
                    out.append(inst)
                    continue
                tname = type(inst).__name__
                if tname == "InstLdweights":
                    sig = (
                        _ap_sig(inst.ins[0]),
                        str(getattr(inst, "perf_mode", None)),
                        str(getattr(inst, "is_transpose", None)),
                        str(getattr(inst, "tile_position", None)),
                        str(getattr(inst, "tile_size", None)),
                    )
                    si = inst.sync_info
                    if sig == last_sig and not (si is not None and si.on_update):
                        if si is not None and si.on_wait:
                            pending.extend(si.on_wait)
                        removed += 1
                        continue
                    last_sig = sig
                elif tname == "InstMatmult":
                    if getattr(inst, "is_transpose", None):
                        last_sig = None
                else:
                    last_sig = None
                if pending:
                    si = inst.sync_info
                    if si is None:
                        inst.sync_info = mybir.SyncInfo(
                            on_wait=list(pending), on_update=[]
                        )
                    else:
                        si.on_wait = list(pending) + list(si.on_wait or [])
                    pending = []
                out.append(inst)
            assert not pending
            bb.instructions = out
    return removed


def _prune_waits(nc):
    """Remove waits that are (a) transitively guaranteed by an earlier wait on
    the same engine, or (b) on a semaphore whose required count is already met
    by the engine's OWN prior updates (in-order engines satisfy these by
    program order). Barrier (eq-imm) waits are never touched."""
    removed = 0
    own = {}         # engine -> {sem_id: count of own updates emitted so far}
    guaranteed = {}  # engine -> {sem_id: max value already waited-for}
    for fn in nc.m.functions:
        for bb in fn.blocks:
            for inst in bb.instructions:
                eng = str(inst.engine)
                g = guaranteed.setdefault(eng, {})
                o = own.setdefault(eng, {})
                si = inst.sync_info
                if si is not None and si.on_wait:
                    keep = []
                    for w in si.on_wait:
                        if w.wait_mode != "sem-ge-imm" or "barrier" in (
                            w.ant_name or ""
                        ):
                            keep.append(w)
                            # eq-imm barrier resets guarantees for this sem
                            g.pop(w.id, None)
                            continue
                        if g.get(w.id, -1) >= w.wait_value:
                            removed += 1
                            continue
                        if o.get(w.id, 0) >= w.wait_value:
                            removed += 1
                            g[w.id] = max(g.get(w.id, -1), w.wait_value)
                            continue
                        g[w.id] = w.wait_value
                        keep.append(w)
                    si.on_wait = keep
                if si is not None and si.on_update:
                    eshort = eng.split(".")[-1] + "_"
                    for u in si.on_update:
                        # only count updates on the engine's OWN queue sem:
                        # DMA/collective completions are async and must not
                        # be treated as satisfied-by-program-order
                        if u.update_mode == "sem-inc" and (
                            u.ant_name or ""
                        ).startswith(eshort):
                            o[u.id] = o.get(u.id, 0) + (u.update_value or 1)
    return removed


# ---------------------------------------------------------------------------

B, S, D, E = 4, 2048, 1024, 64
SC = S // 2          # query/kv rows per core
NT = SC // 128       # 8 local query blocks
NKT = S // 16 // 8   # 16 k-tiles
NKT = S // 128
ND = D // 128        # 8 d-tiles
E1 = E + 1           # V columns + ones column
PITCH = 2 * SC + 128  # 2176: elem pitch of same-parity first-blocks in expt
EPLEN = SC + 8 * PITCH  # padded expt length so both strided views are in-AP

F32 = mybir.dt.float32
F32R = mybir.dt.float32r
BF16 = mybir.dt.bfloat16
EXP = mybir.ActivationFunctionType.Exp

KERNEL_UID = 7000

# parity-independent causal table: t0[kti] = ceil((kti-1)/2) (h=1 ranges,
# a superset of h=0's; surplus is zeroed by the data-driven masks)
T0 = {kti: max(0, -(-(kti - 1) // 2)) for kti in range(NKT)}
T0 = {k: t for k, t in T0.items() if t * 128 < SC}


def _units():
    """Score units: one kti if W>512 else a pair of equal-width ktis."""
    units = []
    ks = sorted(T0)
    i = 0
    while i < len(ks):
        k = ks[i]
        W = SC - T0[k] * 128
        if W > 512 or i + 1 >= len(ks):
            units.append(((k,), T0[k], W))
            i += 1
        else:
            k2 = ks[i + 1]
            assert SC - T0[k2] * 128 == W, (k, k2)
            units.append(((k, k2), T0[k], W))
            i += 2
    return units


UNITS = _units()
NU = len(UNITS)


def _build_nc(loop_reps=None, split=True, timing_mode=False, uid=KERNEL_UID):
    nc = bass.Bass()

    if timing_mode:
        xall = nc.dram_tensor("xall", (128, ND * 3 * SC), BF16)
    else:
        xall = nc.dram_tensor("xall", (128, ND * 3 * SC), BF16,
                              kind="ExternalInput")
    # wall: [128, ND*192]: per (p, a): cols 0:128 = [Wk|Wv][a*128+p, :],
    # 128:192 = Wq[a*128+p, :]
    wall = nc.dram_tensor("wall", (128, ND * 192), BF16, kind="ExternalInput")
    # per-core masks for the first 128-col block of each k-tile range:
    # maskin[0] applies to even ktis, maskin[1] to odd ktis
    maskin = nc.dram_tensor("maskin", (2, 128, 128), BF16, kind="ExternalInput")
    nc.dram_tensor("vtag", (1, uid), F32, kind="ExternalInput")
    out = nc.dram_tensor("out", (E1, SC), F32, kind="ExternalOutput")

    with tile.TileContext(nc) as tc:
        with (
            tc.tile_pool(name="const", bufs=1) as cpool,
            tc.tile_pool(name="ps_u", bufs=3, space="PSUM") as ps_u,
            tc.tile_pool(name="ps_av", bufs=1, space="PSUM") as ps_av,
            tc.tile_pool(name="dram", bufs=1, space="DRAM") as dpool,
        ):
            # ---- one-time constants ----
            w_all = cpool.tile([128, ND, 192], BF16, tag="w_all")
            nc.sync.dma_start(
                out=w_all[:],
                in_=wall[:, :].rearrange("p (a e) -> p a e", e=192),
            )
            w_kv = w_all[:, :, 0:128]
            w_q = w_all[:, :, 128:192]
            masks_sb = cpool.tile([128, 2, 128], BF16, tag="mask")
            nc.sync.dma_start(
                out=masks_sb[:], in_=maskin[:, :, :].rearrange("m p q -> p m q")
            )

            if timing_mode:
                zt = cpool.tile([128, 3 * SC], BF16, tag="zt")
                nc.vector.memset(zt[:], 0.0)
                for zi in range(ND):
                    nc.sync.dma_start(
                        out=xall[:, zi * 3 * SC : (zi + 1) * 3 * SC], in_=zt[:]
                    )

            def init_body(par):
                v1 = cpool.tile([128, NKT, E1], BF16, tag=f"v1{par}")
                ep = cpool.tile([128, EPLEN], BF16, tag=f"ep{par}")
                nc.gpsimd.memset(v1[:, :, E : E + 1], 1.0)
                nc.gpsimd.memset(ep[:], 0.0)

            init_body(0)
            init_body(1)

            def emit_body(par):
                xT = cpool.tile([128, ND, 3, SC], BF16, tag="xT")
                ktb = cpool.tile([E, S], BF16, tag=f"ktb{par}")
                kt = cpool.tile([E, S], F32R, tag=f"kt{par}")
                qt = cpool.tile([E, SC], F32R, tag=f"qt{par}")
                kvt = cpool.tile([128, SC], BF16, tag=f"kvt{par}")
                v1 = cpool.tile([128, NKT, E1], BF16, tag=f"v1{par}")
                ep = cpool.tile([128, EPLEN], BF16, tag=f"ep{par}")
                expt = ep[:, 0 : NKT * SC].rearrange("p (k q) -> p k q", q=SC)
                avsb = cpool.tile([E1, SC], F32, tag=f"avsb{par}")

                # ---- X^T load (host pre-packed; contiguous per partition)
                nc.sync.dma_start(
                    out=xT[:],
                    in_=xall[:, :].rearrange("p (a j s) -> p a j s", a=ND, j=3),
                )

                # ---- projections ----
                # pps_k: parts 0:64 = K^T (64:128 garbage, then overwritten
                # by the Q group); pps_v: parts 64:128 = V^T (0:64 garbage).
                pps_k = ps_u.tile([128, 1024], F32, tag="u")
                pps_v = ps_u.tile([128, 1024], F32, tag="u")
                for dt in range(ND):
                    for c in range(2):
                        nc.tensor.matmul(
                            pps_k[:, c * 512 : (c + 1) * 512],
                            w_kv[:, dt, :],
                            xT[:, dt, 0, c * 512 : (c + 1) * 512],
                            start=(dt == 0), stop=(dt == ND - 1),
                            skip_group_check=True,
                        )
                    for c in range(2):
                        nc.tensor.matmul(
                            pps_v[:, c * 512 : (c + 1) * 512],
                            w_kv[:, dt, :],
                            xT[:, dt, 2, c * 512 : (c + 1) * 512],
                            start=(dt == 0), stop=(dt == ND - 1),
                            skip_group_check=True,
                        )
                nc.scalar.copy(out=kvt[E : 2 * E, :], in_=pps_v[E : 2 * E, :])
                # Q as a sequential group into pps_k parts 64:128 (after all
                # K matmuls; start=True re-zeroes the garbage there)
                for dt in range(ND):
                    for c in range(2):
                        nc.tensor.matmul(
                            pps_k[E : 2 * E, c * 512 : (c + 1) * 512],
                            w_q[:, dt, :],
                            xT[:, dt, 1, c * 512 : (c + 1) * 512],
                            start=(dt == 0), stop=(dt == ND - 1),
                            skip_group_check=True,
                        )
                nc.scalar.copy(out=kvt[0:E, :], in_=pps_k[0:E, :])
                nc.scalar.copy(out=qt[:], in_=pps_k[E : 2 * E, :])

                # ---- K/V exchange: one pairwise AllGather ----
                src_kv = dpool.tile([128, SC], BF16, tag=f"cc_src{par}")
                dst_kv = dpool.tile([2, 128, SC], BF16, tag=f"cc_dst{par}")
                nc.scalar.dma_start(out=src_kv[:, :], in_=kvt[:])
                nc.gpsimd.collective_compute(
                    "AllGather",
                    mybir.AluOpType.bypass,
                    replica_groups=[[0, 1], [2, 3], [4, 5], [6, 7]],
                    ins=[src_kv[:]],
                    outs=[dst_kv[:]],
                )
                nc.gpsimd.dma_start(
                    out=ktb[:].rearrange("e (r s) -> e r s", r=2),
                    in_=dst_kv[:, 0:E, :].rearrange("r e s -> e r s"),
                )
                nc.vector.tensor_copy(out=kt[:], in_=ktb[:])
                # V natural via 2 XBAR transposes + copies into v1
                vscr = cpool.tile([128, 2, 8, E], BF16, tag=f"vscr{par}")
                for r in range(2):
                    nc.sync.dma_start_transpose(
                        out=vscr[:, r, :, :],
                        in_=dst_kv[r, E : 2 * E, :],
                    )
                for r in (1, 0):
                    nc.gpsimd.tensor_copy(
                        out=v1[:, r * 8 : (r + 1) * 8, 0:E], in_=vscr[:, r]
                    )

                # ---- scores + exp, REVERSE unit order ----
                def unit_scores(u):
                    ktis, t0, W = UNITS[u]
                    sps = ps_u.tile([128, 1024], F32, tag="u")
                    base = 0 if len(ktis) == 1 else 512 - W
                    for j, kti in enumerate(ktis):
                        lo = base + j * W
                        off = 0
                        while off < W:
                            w_ = min(512, W - off)
                            nc.tensor.matmul(
                                sps[:, lo + off : lo + off + w_],
                                kt[:, kti * 128 : (kti + 1) * 128],
                                qt[:, t0 * 128 + off : t0 * 128 + off + w_],
                                start=True, stop=True,
                                skip_group_check=True,
                            )
                            off += w_
                    return sps

                def unit_exp(u, sps):
                    ktis, t0, W = UNITS[u]
                    k0 = ktis[0]
                    nk = len(ktis)
                    base = 0 if nk == 1 else 512 - W
                    nc.scalar.activation(
                        expt[:, k0 : k0 + nk, t0 * 128 : t0 * 128 + W],
                        sps[:, base : base + nk * W].rearrange(
                            "p (j q) -> p j q", j=nk
                        ),
                        EXP,
                    )

                batches = []
                us = list(range(NU - 1, -1, -1))
                for i in range(0, NU, 3):
                    batches.append(us[i : i + 3])
                for batch in batches:
                    tiles = [(u, unit_scores(u)) for u in batch]
                    for u, sps in reversed(tiles):
                        unit_exp(u, sps)

                # ---- data-driven boundary masks: 2 strided DVE multiplies
                # even ktis 2j: first block at elem offset j*PITCH
                # odd ktis 2j+1: first block at elem offset SC + j*PITCH
                for m, base in ((1, SC), (0, 0)):  # odd first, even LAST
                    blk = (
                        ep[:, base : base + 8 * PITCH]
                        .rearrange("p (t r) -> p t r", r=PITCH)[:, :, 0:128]
                    )
                    nc.vector.tensor_mul(
                        blk,
                        blk,
                        masks_sb[:, m : m + 1, :].broadcast_to([128, 8, 128]),
                    )

                # ---- AV: one [65,1024] psum, kti ASCENDING, nested ranges
                av = ps_av.tile([E1, SC], F32, tag="av")
                ks = sorted(T0)
                for i, kti in enumerate(ks):
                    lo = T0[kti] * 128
                    first, last = i == 0, i == len(ks) - 1
                    if lo < 512:
                        nc.tensor.matmul(
                            av[:, lo:512],
                            v1[:, kti, :],
                            expt[:, kti, lo:512],
                            start=first, stop=last,
                            skip_group_check=True,
                        )
                    nc.tensor.matmul(
                        av[:, 512:SC],
                        v1[:, kti, :],
                        expt[:, kti, 512:SC],
                        start=first, stop=last,
                        skip_group_check=True,
                    )

                # ---- store numerator+denominator; host divides ----
                nc.vector.tensor_copy(out=avsb[:], in_=av[:])
                nc.sync.dma_start(out=out[:, :], in_=avsb[:])

            for _rep in range(1 if loop_reps is None else loop_reps):
                emit_body(_rep % 2)

    _dedup_ldweights(nc)
    _prune_waits(nc)
    if split:
        _split_sync_waits(nc)
    return nc


_CACHE = {}


def _get_nc():
    if "nc" not in _CACHE:
        _CACHE["nc"] = _build_nc()
    return _CACHE["nc"]


def _host_masks(h):
    """maskin[0] -> even ktis, maskin[1] -> odd ktis. Same-parity ktis get
    the tril boundary mask; other-parity ktis get all-ones for h=1 (their
    first block is fully causal there) or all-zeros for h=0 (fully above
    the diagonal)."""
    ki = np.arange(128)[:, None]
    qi = np.arange(128)[None, :]
    tril = (ki <= qi).astype(np.float32)
    other = np.full((128, 128), 1.0 if h == 1 else 0.0, np.float32)
    m = np.empty((2, 128, 128), np.float32)
    m[h] = tril
    m[1 - h] = other
    return m.astype(ml_dtypes.bfloat16)


def kernel(**inputs):
    xq_full = np.asarray(inputs["inputs_for_queries"], dtype=np.float32).astype(
        ml_dtypes.bfloat16
    )
    xk_full = np.asarray(inputs["inputs_for_keys"], dtype=np.float32).astype(
        ml_dtypes.bfloat16
    )
    xv_full = np.asarray(inputs["inputs_for_values"], dtype=np.float32).astype(
        ml_dtypes.bfloat16
    )
    wq = (np.asarray(inputs["Weight_Q"], dtype=np.float32) * 0.125).astype(
        ml_dtypes.bfloat16
    )
    wk = np.asarray(inputs["Weight_K"], dtype=np.float32).astype(ml_dtypes.bfloat16)
    wv = np.asarray(inputs["Weight_V"], dtype=np.float32).astype(ml_dtypes.bfloat16)
    wkv = np.concatenate([wk, wv], axis=1)  # [D, 128]
    wall = np.concatenate(
        [wkv.reshape(ND, 128, 128), wq.reshape(ND, 128, 64)], axis=2
    )
    wall = np.ascontiguousarray(wall.transpose(1, 0, 2).reshape(128, ND * 192))

    masks_h = [_host_masks(h) for h in (0, 1)]

    nc = _get_nc()

    in_maps = []
    for c in range(8):
        b, h = c // 2, c % 2
        rows = np.concatenate(
            [np.arange((2 * t + h) * 128, (2 * t + h + 1) * 128) for t in range(NT)]
        )
        xk_h = xk_full[b][h * SC : (h + 1) * SC]  # [SC, D]
        xq_h = xq_full[b][rows]
        xv_h = xv_full[b][h * SC : (h + 1) * SC]
        xs = np.stack(
            [x.T.reshape(ND, 128, SC) for x in (xk_h, xq_h, xv_h)], axis=2
        )  # [ND, 128, 3, SC]
        xall = np.ascontiguousarray(
            xs.transpose(1, 0, 2, 3).reshape(128, ND * 3 * SC)
        )
        in_maps.append(
            {
                "xall": xall,
                "wall": wall,
                "maskin": masks_h[h],
                "vtag": np.zeros((1, KERNEL_UID), np.float32),
            }
        )

    trace = bool(int(os.environ.get("KERNEL_TRACE", "0")))
    res = run_bass_kernel_spmd(nc, in_maps, core_ids=list(range(8)), trace=trace)
    if trace:
        _CACHE["last_results"] = res

    out_full = np.empty((B, S, E), dtype=np.float32)
    for c in range(8):
        b, h = c // 2, c % 2
        oc = res.results[c]["out"]  # [65, SC]
        numer = oc[0:E, :]
        den = oc[E, :]
        for t in range(NT):
            g = 2 * t + h
            sl = slice(t * 128, (t + 1) * 128)
            out_full[b, g * 128 : (g + 1) * 128] = (numer[:, sl] / den[sl]).T
    return out_full
